# revision 1
# baseline (speedup 1.0000x reference)
"""Trainium2 Bass kernel for nn_ExtSummModel (extractive summarization).

Data-parallel over docs: 8 cores x 4 docs, single SPMD launch. Cross-core
exchange: AllGather of GRU final hiddens (hidden.reshape(B, 2H) makes
doc_vec[b] depend on docs 2b, 2b+1).

Schedule: embedding gathers (Pool) + fp32r projections (PE) are emitted
interleaved with the bidirectional GRU so they pipeline under it; the srep
DRAM bounce + srepT materialization happen during the GRU's second half;
the final-hidden AllGather overlaps the topic phase; attention runs in
fp32r with the srep half of each accumulation first.

xT/giT columns are ordered by GATHER order (blocks 0,7,1,6,2,5,3,4) so that
each pipelined block pair is column-adjacent; ob/srepT stay time-ordered.

Self-contained: hardcodes shapes; host side only shards/packs numpy inputs.
"""
import os
import sys

sys.path.insert(0, "/opt/trn_rl_repo")

import numpy as np
import concourse.bacc as bacc
import concourse.bass as bass
import concourse.mybir as mybir
import concourse.tile as tile
from concourse.bass_utils import run_bass_kernel_spmd
from concourse.masks import make_identity

B, S, L, E, H, T, V, D = 32, 256, 24, 300, 256, 16, 50000, 256
NC = 8
BD = B // NC          # 4 docs per core
SP = S + 2            # padded sentence axis
H2, H3, H4 = 2 * H, 3 * H, 4 * H
NBLK = 8              # sentence blocks of 32 s x 4 docs = 128 rows
NW = NBLK * L         # 192 gather columns
EKC = [128, 128, 44]  # E=300 row chunks (chunk2 gets a ones row at 44)
BLK_ORDER = [0, 7, 1, 6, 2, 5, 3, 4]          # gather order (ord -> block)
ORD_OF = {b: o for o, b in enumerate(BLK_ORDER)}

f32 = mybir.dt.float32
f32r = mybir.dt.float32r
i32 = mybir.dt.int32
AF = mybir.ActivationFunctionType
OP = mybir.AluOpType

_BUILT = None


def _gcol(t):
    """giT/xT column block index for GRU time t (gather-order layout)."""
    return ORD_OF[t // 32] * 32 + (t % 32)


def _emit(tc, nc, ein, logits, dbg):
    from contextlib import ExitStack
    ctx = ExitStack()
    res = ctx.enter_context(tc.tile_pool(name="res", bufs=1, side="left"))
    dram = ctx.enter_context(tc.tile_pool(name="dram", bufs=1, space="DRAM"))

    # ---------------- residents ----------------
    ident = res.tile([128, 128], f32, tag="ident", name="ident")
    make_identity(nc, ident[:])

    wf = [res.tile([128, H3], f32, tag=f"wf{k}", name=f"wf{k}") for k in range(2)]
    wb = [res.tile([128, H3], f32, tag=f"wb{k}", name=f"wb{k}") for k in range(2)]
    for k in range(2):
        nc.sync.dma_start(wf[k][:], ein["whhT_f"].ap()[k * 128:(k + 1) * 128, :])
        nc.sync.dma_start(wb[k][:], ein["whhT_b"].ap()[k * 128:(k + 1) * 128, :])
    wdir = {"f": wf, "b": wb}

    bhhn = {}
    for d in ("f", "b"):
        r = res.tile([1, H], f32, tag=f"bhhn{d}", name=f"bhhn{d}")
        nc.sync.dma_start(r[:], ein[f"bhh_{d}"].ap().rearrange("(a n) -> a n", a=1)[:, 2 * H:3 * H])
        bhhn[d] = r
    onesr = res.tile([1, 256], f32, tag="onesr", name="onesr")
    nc.gpsimd.memset(onesr[:], 1.0)
    ones4 = onesr[:, 0:BD]
    onesr_r = res.tile([1, 256], f32r, tag="onesrr", name="onesrr")
    nc.vector.tensor_copy(onesr_r[:], onesr[:])
    identr = res.tile([128, 128], f32r, tag="identr", name="identr")
    nc.vector.tensor_copy(identr[:], ident[:])
    hinit = res.tile([128, 8], f32, tag="hinit", name="hinit")
    nc.gpsimd.memset(hinit[:], 0.0)

    vatt = res.tile([128, 8], f32r, tag="vatt", name="vatt")
    nc.sync.dma_start(vatt[:], ein["v_att"].ap().rearrange("(m p) o -> p (m o)", p=128).bitcast(f32r))
    wout = res.tile([128, 2], f32r, tag="wout", name="wout")
    nc.sync.dma_start(wout[:], ein["w_out"].ap().rearrange("(m p) o -> p (m o)", p=128).bitcast(f32r))
    bdna = res.tile([128, 2], f32, tag="bdna", name="bdna")
    nc.sync.dma_start(bdna[:], ein["b_dna"].ap().rearrange("(m p) -> p m", p=128))
    bout = res.tile([1, 1], f32, tag="bout", name="bout")
    nc.sync.dma_start(bout[:], ein["b_out"].ap().rearrange("(a o) -> a o", a=1))


    p5s_cm = tc.tile_pool(name="p5s", bufs=1, side="left")
    p5s = p5s_cm.__enter__()
    p4w_cm = tc.tile_pool(name="p4w", bufs=2, side="left")
    p4w = p4w_cm.__enter__()
    ob_cm = tc.tile_pool(name="obpool", bufs=1, side="left")
    obp = ob_cm.__enter__()
    ob = {"f": obp.tile([128, S * 8], f32, tag="obf", name="obf"),
          "b": obp.tile([128, S * 8], f32, tag="obb", name="obb")}
    gi_cm = tc.tile_pool(name="gipool", bufs=1, side="left")
    gip = gi_cm.__enter__()
    giT = {"f": gip.tile([128, S * 24], f32, tag="gif", name="gif"),
           "b": gip.tile([128, S * 24], f32, tag="gib", name="gib")}

    srepT = [res.tile([128, S * BD], f32r, tag=f"srepT{k}", name=f"srepT{k}") for k in range(4)]
    topicrepT = [res.tile([128, S * BD], f32r, tag=f"trep{c}", name=f"trep{c}") for c in range(4)]
    dvT = [res.tile([128, BD], f32r, tag=f"dvT{c}", name=f"dvT{c}") for c in range(4)]

    srep = dram.tile([BD * SP, H2], f32, name="srep")
    cc_in = dram.tile([8, H], f32, name="cc_in")
    cc_out = dram.tile([8 * NC, H], f32, name="cc_out")

    # ---------------- early pools (gather / transpose / projection) -------
    early_cm = tc.tile_pool(name="early", bufs=1, side="right")
    early = early_cm.__enter__()
    wid_sb = early.tile([128, NW], i32, tag="wid", name="wid")
    nc.sync.dma_start(wid_sb[:], ein["wid"].ap())
    wih = {}
    wbias = {}
    for d in ("f", "b"):
        for k in range(3):
            t_ = early.tile([EKC[k], H3], f32r, tag=f"wih{d}{k}", name=f"wih{d}{k}")
            nc.sync.dma_start(t_[:], ein[f"wihT_{d}"].ap()[128 * k:128 * k + EKC[k], :].bitcast(f32r))
            wih[(d, k)] = t_
        wb_ = early.tile([1, H3], f32r, tag=f"wbias{d}", name=f"wbias{d}")
        nc.sync.dma_start(wb_[:], ein[f"wihT_{d}"].ap()[E:E + 1, :].bitcast(f32r))
        wbias[d] = wb_
    xT = [early.tile([EKC[k], S * BD], f32r, tag=f"xT{k}", name=f"xT{k}")
          for k in range(3)]

    gat_cm = tc.tile_pool(name="gat", bufs=3, side="right")
    gat = gat_cm.__enter__()
    pbig_cm = tc.tile_pool(name="pbig", bufs=4, space="PSUM")
    pbig = pbig_cm.__enter__()

    def emit_gather(ordi):
        xs = gat.tile([128, E], f32, tag="xs", name="xs")
        for l in range(L):
            nc.gpsimd.indirect_dma_start(
                out=xs[:], out_offset=None, in_=ein["emb"].ap(),
                in_offset=bass.IndirectOffsetOnAxis(
                    ap=wid_sb[:, ordi * L + l:ordi * L + l + 1], axis=0),
                compute_op=(OP.bypass if l == 0 else OP.add))
        return xs

    def emit_xt(ordi, xs, copy_eng):
        for k in range(3):
            ps = pbig.tile([128, 256], f32, tag="big", name="bigx")
            nc.tensor.transpose(ps[0:EKC[k], 0:128], xs[:, 128 * k:128 * k + EKC[k]], ident[:])
            cp(copy_eng + k, xT[k][0:EKC[k], ordi * 128:(ordi + 1) * 128],
               ps[0:EKC[k], 0:128])

    def emit_proj(pair_i, engs):
        c0 = pair_i * 256
        for d in ("f", "b"):
            for m in range(6):
                ps = pbig.tile([128, 256], f32, tag="big", name="bigp")
                for k in range(3):
                    nc.tensor.matmul(
                        ps[:], wih[(d, k)][0:EKC[k], m * 128:(m + 1) * 128],
                        xT[k][0:EKC[k], c0:c0 + 256],
                        start=(k == 0), stop=False)
                nc.tensor.matmul(ps[:], wbias[d][:, m * 128:(m + 1) * 128],
                                 onesr_r[:], start=False, stop=True)
                out_ap = giT[d][:].rearrange("p (s md) -> p s md", md=24)[
                    :, pair_i * 64:(pair_i + 1) * 64, m * 4:(m + 1) * 4]
                in_ap = ps[:].rearrange("p (s dd) -> p s dd", dd=4)
                cp(m, out_ap, in_ap)

    def cp(i, out, in_):
        # i: 0 -> DVE copy, 1 -> Act copy
        if i % 2 == 0:
            nc.vector.tensor_copy(out, in_)
        else:
            nc.scalar.activation(out, in_, AF.Identity)

    ceng = [0, 1]

    # prologue: pair 0 fully
    xs0 = emit_gather(0)
    xs1 = emit_gather(1)
    emit_xt(0, xs0, 0)
    emit_xt(1, xs1, 1)
    emit_proj(0, ceng)
    xs_cache = {}

    # topic mask prep consts (pool opened earlier)
    iota_i = p5s.tile([128, S], i32, tag="iotai", name="iotai")
    nc.gpsimd.iota(iota_i[:], pattern=[[1, S]], base=0, channel_multiplier=0)
    iota_f = p5s.tile([128, S], f32, tag="iotaf", name="iotaf")
    nc.vector.tensor_copy(iota_f[:], iota_i[:])
    tenf = p5s.tile([128, 1], f32, tag="tenf", name="tenf")
    nc.sync.dma_start(tenf[:], ein["tenf"].ap())
    tepf = p5s.tile([128, 1], f32, tag="tepf", name="tepf")
    nc.sync.dma_start(tepf[:], ein["tepf"].ap())
    tof = p5s.tile([128, 4], i32, tag="tof", name="tof")
    nc.sync.dma_start(tof[:], ein["tof"].ap().rearrange("ty p -> p ty"))
    oh = p5s.tile([128, S], f32r, tag="oh", name="oh")

    def emit_topic_prep(_):
        e_m = p5s.tile([128, S], f32, tag="em", name="em")
        nc.vector.tensor_scalar(e_m[:], iota_f[:], tenf[:, 0:1], None, op0=OP.is_lt)
        ep_m = p5s.tile([128, S], f32, tag="epm", name="epm")
        nc.vector.tensor_scalar(ep_m[:], iota_f[:], tepf[:, 0:1], None, op0=OP.is_lt)
        t_ = p5s.tile([128, S], f32, tag="ohtmp", name="ohtmp")
        nc.vector.tensor_mul(t_[:], e_m[:], ep_m[:])
        nc.vector.tensor_sub(oh[:], e_m[:], t_[:])

    # ---------------- late pool (attention weights; created mid-GRU) ------
    late_state = {}

    def emit_late_open():
        late_state["cm"] = tc.tile_pool(name="late", bufs=1, side="right")
        late = late_state["cm"].__enter__()
        late_state["watt"] = [late.tile([128, H4], f32r, tag=f"watt{k}", name=f"watt{k}")
                              for k in range(8)]
        late_state["wdna"] = [late.tile([128, D], f32r, tag=f"wdna{k}", name=f"wdna{k}")
                             for k in range(8)]
        for k in range(8):
            nc.sync.dma_start(late_state["watt"][k][:],
                              ein["w_att"].ap()[k * 128:(k + 1) * 128, :].bitcast(f32r))
        for k in range(8):
            nc.sync.dma_start(late_state["wdna"][k][:],
                              ein["w_dna"].ap()[k * 128:(k + 1) * 128, :].bitcast(f32r))

    # ---------------- srep bounce + srepT pieces --------------------------
    zrow = p4w.tile([1, H2], f32, tag="zrow", name="zrow")
    nc.gpsimd.memset(zrow[:], 0.0)
    for d in range(BD):
        nc.sync.dma_start(srep[d * SP:d * SP + 1, :], zrow[:])
        nc.sync.dma_start(srep[d * SP + SP - 1:d * SP + SP, :], zrow[:])

    def emit_bounce(dirn, sb_, copy_eng):
        half = 0 if dirn == "f" else 1
        for kk in range(2):
            k4 = kk if dirn == "f" else 2 + kk
            src = srepT[k4][:, sb_ * 128:(sb_ + 1) * 128]
            ps2 = pbig.tile([128, 256], f32, tag="big", name="bigb")
            nc.tensor.matmul(ps2[:, 0:128].bitcast(f32r), src, identr[:],
                             is_transpose=True, start=True, stop=True)
            st2 = p4w.tile([128, 128], f32, tag="s2st", name="s2st")
            cp(copy_eng + kk, st2[:], ps2[:, 0:128])
            dst = srep[:].rearrange("(d sp) c -> sp d c", d=BD)[
                1 + sb_ * 32: 1 + (sb_ + 1) * 32, :,
                half * H + kk * 128: half * H + kk * 128 + 128]
            nc.sync.dma_start(dst, st2[:])

    def emit_srepT(dirn, sb_, eng):
        for kk in range(2):
            k4 = kk if dirn == "f" else 2 + kk
            src = ob[dirn][:].rearrange("p (s kd) -> p s kd", kd=8)[
                :, sb_ * 32:(sb_ + 1) * 32, kk * 4:(kk + 1) * 4]
            cp(eng + kk,
               srepT[k4][:].rearrange("p (s dd) -> p s dd", dd=4)[
                   :, sb_ * 32:(sb_ + 1) * 32, :], src)

    # ---------------- GRU ----------------
    p3w_cm = tc.tile_pool(name="p3w", bufs=3, side="left")
    p3w = p3w_cm.__enter__()
    p3p_cm = tc.tile_pool(name="p3p", bufs=2, space="PSUM")
    p3p = p3p_cm.__enter__()

    # GRU state: h2 kept as two addends t1 = z*h_prev, t2 = (1-z)*n, consumed
    # separately by the Whh matmuls (PSUM accumulates), so the h2 add is off
    # the serial ring. Both directions share one PSUM gate tile (f cols 0:24,
    # b cols 24:48) so sigmoid/tanh/muls run as single fused instructions.
    prev = {}
    DOFF = {"f": 0, "b": 24}

    def gru_pe_head(d, t, gh):
        # accumulation-group openers: gi identity add (rz) + bhh_n broadcast
        gc = _gcol(t) * 24
        o = DOFF[d]
        nc.tensor.matmul(gh[:, o:o + 16], ident[:], giT[d][:, gc:gc + 16],
                         start=True, stop=False, skip_group_check=True)
        for m in (4, 5):
            c0 = o + 16 + (m - 4) * 4
            nc.tensor.matmul(gh[:, c0:c0 + 4],
                             bhhn[d][:, (m - 4) * 128:(m - 3) * 128],
                             ones4, start=True, stop=False, skip_group_check=True)

    def gru_pe_h(d, first, gh, which, ms):
        # Whh contributions from one h2 addend over gate blocks ms
        o = DOFF[d]
        hp = hinit[:, 0:8] if first else \
            prev[which][:, DOFF[d] // 3:DOFF[d] // 3 + 8]
        for m in ms:
            c0 = o + m * 4
            for kk in range(2):
                nc.tensor.matmul(gh[:, c0:c0 + 4],
                                 wdir[d][kk][:, m * 128:(m + 1) * 128],
                                 hp[:, kk * 4:(kk + 1) * 4], start=False,
                                 stop=(which == "t2" and kk == 1),
                                 skip_group_check=True)

    ghv = lambda gh, a, b: gh[:].rearrange("p (dd c) -> p dd c", dd=2)[:, :, a:b]

    # interleave: window k (iters 32k..32k+32) prepares pair k+1
    def hook(j):
        if j < 96:
            k, r = j // 32, j % 32
            if r == 1:
                xs_cache[2 * k + 2] = emit_gather(2 * k + 2)
            elif r == 12:
                xs_cache[2 * k + 3] = emit_gather(2 * k + 3)
            elif r == 24:
                emit_xt(2 * k + 2, xs_cache.pop(2 * k + 2), ceng[k % 2])
            elif r == 26:
                emit_xt(2 * k + 3, xs_cache.pop(2 * k + 3), ceng[(k + 1) % 2])
            elif r == 28:
                emit_proj(k + 1, ceng)
        if j == 95:
            gat_cm.__exit__(None, None, None)
            early_cm.__exit__(None, None, None)
            emit_late_open()
        elif j == 100:
            emit_topic_prep(0)
        # bounce/srepT: f block b ready after iter 32b+31; b block b after 255-32b
        if 129 <= j < 193 and (j - 129) % 8 == 0:
            i = (j - 129) // 8
            pieces = [("f", 0), ("b", 7), ("f", 1), ("b", 6),
                      ("f", 2), ("b", 5), ("f", 3), ("b", 4)]
            dirn, bb = pieces[i]
            emit_srepT(dirn, bb, ceng[(i + 1) % 2])
            emit_bounce(dirn, bb, ceng[i % 2])
        for thr, dirn, bb in ((195, "f", 4), (197, "b", 3), (211, "f", 5),
                              (213, "b", 2), (227, "f", 6), (229, "b", 1)):
            if j == thr:
                emit_srepT(dirn, bb, ceng[(bb + 1) % 2])
                emit_bounce(dirn, bb, ceng[bb % 2])

    for j in range(S):
        tf, tb = j, S - 1 - j
        first = j == 0
        gh = p3p.tile([128, 48], f32, tag="gh", name="gh")
        gru_pe_head("f", tf, gh)
        gru_pe_head("b", tb, gh)
        RZ, HN = (0, 1, 2, 3), (4, 5)
        gru_pe_h("f", first, gh, "t1", RZ)
        gru_pe_h("b", first, gh, "t1", RZ)
        gru_pe_h("f", first, gh, "t2", RZ)
        gru_pe_h("b", first, gh, "t2", RZ)
        gru_pe_h("f", first, gh, "t1", HN)
        gru_pe_h("b", first, gh, "t1", HN)
        gru_pe_h("f", first, gh, "t2", HN)
        gru_pe_h("b", first, gh, "t2", HN)
        rz = p3w.tile([128, 32], f32, tag="rz", name="rz")
        nc.scalar.activation(rz[:].rearrange("p (dd c) -> p dd c", dd=2),
                             ghv(gh, 0, 16), AF.Sigmoid)
        omz = p3w.tile([128, 16], f32, tag="omz", name="omz")
        nc.vector.tensor_scalar(
            omz[:].rearrange("p (dd c) -> p dd c", dd=2),
            rz[:].rearrange("p (dd c) -> p dd c", dd=2)[:, :, 8:16],
            -1.0, 1.0, op0=OP.mult, op1=OP.add)
        rn = p3w.tile([128, 16], f32, tag="rn", name="rn")
        nc.vector.tensor_mul(rn[:].rearrange("p (dd c) -> p dd c", dd=2),
                             rz[:].rearrange("p (dd c) -> p dd c", dd=2)[:, :, 0:8],
                             ghv(gh, 16, 24))
        npre = p3w.tile([128, 16], f32, tag="npre", name="npre")
        nc.vector.tensor_add(npre[:, 0:8], rn[:, 0:8],
                             giT["f"][:, _gcol(tf) * 24 + 16:_gcol(tf) * 24 + 24])
        nc.vector.tensor_add(npre[:, 8:16], rn[:, 8:16],
                             giT["b"][:, _gcol(tb) * 24 + 16:_gcol(tb) * 24 + 24])
        nt = p3w.tile([128, 16], f32, tag="nt", name="nt")
        nc.scalar.activation(nt[:], npre[:], AF.Tanh)
        t2 = p3w.tile([128, 16], f32, tag="t2", name="t2")
        nc.vector.tensor_mul(t2[:], omz[:], nt[:])
        t1 = p3w.tile([128, 16], f32, tag="t1", name="t1")
        hpf = hinit[:, 0:8] if first else ob["f"][:, (tf - 1) * 8:tf * 8]
        hpb = hinit[:, 0:8] if first else ob["b"][:, (tb + 1) * 8:(tb + 2) * 8]
        nc.vector.tensor_mul(t1[:, 0:8], rz[:, 8:16], hpf)
        nc.vector.tensor_mul(t1[:, 8:16], rz[:, 24:32], hpb)
        nc.vector.tensor_add(ob["f"][:, tf * 8:(tf + 1) * 8], t1[:, 0:8], t2[:, 0:8])
        nc.vector.tensor_add(ob["b"][:, tb * 8:(tb + 1) * 8], t1[:, 8:16], t2[:, 8:16])
        prev["t1"], prev["t2"] = t1, t2
        if dbg and j == 0:
            nc.sync.dma_start(dbg["d_rz0"].ap(), rz[:])
            nc.sync.dma_start(dbg["d_nt0"].ap(), nt[:])
            nc.sync.dma_start(dbg["d_t20"].ap(), t2[:])
            nc.sync.dma_start(dbg["d_om0"].ap(), omz[:])
        hook(j)

    emit_srepT("f", 7, 1)
    emit_bounce("f", 7, 0)
    emit_srepT("b", 0, 0)
    emit_bounce("b", 0, 1)

    if dbg:
        nc.sync.dma_start(dbg["d_gif"].ap(), giT["f"][:])
        nc.sync.dma_start(dbg["d_obf"].ap(), ob["f"][:])
        nc.sync.dma_start(dbg["d_obb"].ap(), ob["b"][:])
        nc.sync.dma_start(dbg["d_srep"].ap(), srep[:])

    p3p_cm.__exit__(None, None, None)
    p3w_cm.__exit__(None, None, None)
    gi_cm.__exit__(None, None, None)

    # ---------------- final hiddens -> AllGather -> doc_vec ----------------
    for dirn, off, t0 in (("f", 0, S - 1), ("b", BD, 0)):
        for kk in range(2):
            dst = cc_in[off:off + BD, kk * 128:(kk + 1) * 128].rearrange("a c -> c a")
            nc.sync.dma_start(dst, ob[dirn][:, t0 * 8 + kk * 4:t0 * 8 + (kk + 1) * 4])
    nc.gpsimd.collective_compute(
        "AllGather", OP.bypass, replica_groups=[list(range(NC))],
        ins=[cc_in.opt()], outs=[cc_out.opt()])
    ob_cm.__exit__(None, None, None)

    dvr_sb = p4w.tile([8, 1], i32, tag="dvr", name="dvr")
    nc.sync.dma_start(dvr_sb[:], ein["dvrows"].ap())
    dvraw = p4w.tile([8, H], f32, tag="dvraw", name="dvraw")
    nc.gpsimd.indirect_dma_start(
        out=dvraw[:], out_offset=None, in_=cc_out[:],
        in_offset=bass.IndirectOffsetOnAxis(ap=dvr_sb[:, 0:1], axis=0))
    tps = []
    for half in range(2):
        ps = pbig.tile([128, 256], f32, tag="big", name="bigd")
        nc.tensor.transpose(ps[:, 0:8], dvraw[:, half * 128:(half + 1) * 128],
                            ident[0:8, 0:8])
        tps.append(ps)
    for c in range(4):
        src = tps[c % 2][:, 0:8].rearrange("p (b two) -> p two b", two=2)[:, c // 2, :]
        nc.vector.tensor_copy(dvT[c][:], src)
    docrepT = [res.tile([128, S * BD], f32r, tag=f"drep{c}", name=f"drep{c}")
               for c in range(4)]
    for c in range(4):
        for d in range(BD):
            nc.vector.tensor_copy(
                docrepT[c][:].rearrange("p (s dd) -> p dd s", dd=BD)[:, d, :],
                dvT[c][:, d:d + 1].to_broadcast([128, S]))
    pbig_cm.__exit__(None, None, None)
    p4w_cm.__exit__(None, None, None)

    # ---------------- topics: gather boundaries, expand -------------------
    with tc.tile_pool(name="p5w", bufs=2) as p5w, \
         tc.tile_pool(name="p5p", bufs=2, space="PSUM") as p5p:
        # per type ty (en, sm1, st, ep1): gather srep rows at partitions d*32+t
        g4 = []
        for ty in range(4):
            g_ = p5w.tile([128, H2], f32, tag=f"g{ty}", name=f"g{ty}")
            nc.gpsimd.indirect_dma_start(
                out=g_[:], out_offset=None, in_=srep[:],
                in_offset=bass.IndirectOffsetOnAxis(ap=tof[:, ty:ty + 1], axis=0))
            g4.append(g_)
        tmat = p5w.tile([128, H2], f32r, tag="tmat", name="tmat")
        nc.vector.tensor_sub(tmat[:, 0:H], g4[0][:, 0:H], g4[1][:, 0:H])
        nc.vector.tensor_sub(tmat[:, H:], g4[2][:, H:], g4[3][:, H:])
        # PE operands need base partition in {0,32,64}: bounce doc 3 to base 0
        tmat3 = p5w.tile([T, H2], f32r, tag="tmat3", name="tmat3")
        nc.sync.dma_start(tmat3[:], tmat[96:96 + T, :])
        oh3 = p5w.tile([T, S], f32r, tag="oh3", name="oh3")
        nc.sync.dma_start(oh3[:], oh[96:96 + T, :])
        for c in range(4):
            for d in range(BD):
                b0 = d * 32
                lhs = tmat[b0:b0 + T, c * 128:(c + 1) * 128] if d < 3 else \
                    tmat3[:, c * 128:(c + 1) * 128]
                rhs = oh[b0:b0 + T, :] if d < 3 else oh3[:]
                ps = p5p.tile([128, S], f32, tag="trp", name="trp")
                nc.tensor.matmul(ps[:], lhs, rhs, start=True, stop=True)
                nc.vector.tensor_copy(
                    topicrepT[c][:].rearrange("p (s dd) -> p dd s", dd=BD)[:, d, :],
                    ps[:])

    if dbg:
        nc.sync.dma_start(dbg["d_trep0"].ap(), topicrepT[0][:].bitcast(f32))
    p5s_cm.__exit__(None, None, None)

    # ---------------- attention ----------------
    scores_w = {}
    K_ORDER = [4, 5, 6, 7, 0, 1, 2, 3]  # srep half first (overlaps collective)
    with (
        tc.tile_pool(name="p6w", bufs=2) as p6w,
        tc.tile_pool(name="p6one", bufs=1) as p6one,
        tc.tile_pool(name="p6s", bufs=3) as p6s,
    ):
        watt, wdna = late_state["watt"], late_state["wdna"]
        for kind in ("ds", "ts"):
            with (
                tc.tile_pool(name=f"p6pa{kind}", bufs=2, space="PSUM") as p6pa,
                tc.tile_pool(name=f"p6ps{kind}", bufs=1, space="PSUM") as p6ps,
            ):
                sc_ps = [p6ps.tile([1, 512], f32, tag=f"scp{nh}", name=f"scp{nh}")
                         for nh in range(2)]
                # software-pipelined: v_att matmul of chunk m is emitted after
                # chunk m+1's accumulation so the PE never stalls on the tanh
                pts = {}
                for m in range(8):
                    pm = [p6pa.tile([128, 512], f32, tag=f"attp{nh}", name=f"attp{nh}")
                          for nh in range(2)]
                    for ki, k in enumerate(K_ORDER):
                        for nh in range(2):
                            if k >= 4:
                                rhs = srepT[k - 4][:, nh * 512:(nh + 1) * 512]
                            elif kind == "ds":
                                rhs = docrepT[k][:, nh * 512:(nh + 1) * 512]
                            else:
                                rhs = topicrepT[k][:, nh * 512:(nh + 1) * 512]
                            nc.tensor.matmul(pm[nh][:],
                                             watt[k][:, m * 128:(m + 1) * 128],
                                             rhs, start=(ki == 0), stop=(ki == 7))
                    pt = p6w.tile([128, H4], f32r, tag="ptanh", name="ptanh", bufs=3)
                    for nh in range(2):
                        nc.scalar.activation(pt[:, nh * 512:(nh + 1) * 512],
                                             pm[nh][:], AF.Tanh)
                    pts[m] = pt
                    if m >= 1:
                        for nh in range(2):
                            nc.tensor.matmul(sc_ps[nh][:], vatt[:, m - 1:m],
                                             pts[m - 1][:, nh * 512:(nh + 1) * 512],
                                             start=(m - 1 == 0), stop=False)
                        del pts[m - 1]
                for nh in range(2):
                    nc.tensor.matmul(sc_ps[nh][:], vatt[:, 7:8],
                                     pts[7][:, nh * 512:(nh + 1) * 512],
                                     start=False, stop=True)
                sc = p6one.tile([1, S * BD], f32, tag=f"sc{kind}", name=f"sc{kind}")
                for nh in range(2):
                    nc.vector.tensor_copy(sc[:, nh * 512:(nh + 1) * 512], sc_ps[nh][:])
            w_ = p6one.tile([1, S * BD], f32r, tag=f"w{kind}", name=f"w{kind}")
            for d in range(BD):
                sl = sc[:].rearrange("o (s dd) -> o dd s", dd=BD)[:, d, :]
                wl = w_[:].rearrange("o (s dd) -> o dd s", dd=BD)[:, d, :]
                mx = p6s.tile([1, 1], f32, tag="mx", name="mx")
                nc.vector.reduce_max(mx[:], sl, axis=mybir.AxisListType.X)
                sh = p6s.tile([1, S], f32, tag="sh", name="sh")
                nc.vector.tensor_scalar(sh[:], sl, mx[:, 0:1], None, op0=OP.subtract)
                ex = p6s.tile([1, S], f32, tag="ex", name="ex")
                nc.scalar.activation(ex[:], sh[:], AF.Exp)
                sm = p6s.tile([1, 1], f32, tag="sm", name="sm")
                nc.vector.reduce_sum(sm[:], ex[:], axis=mybir.AxisListType.X)
                rc = p6s.tile([1, 1], f32, tag="rc", name="rc")
                nc.vector.reciprocal(rc[:], sm[:])
                nc.vector.tensor_scalar(wl, ex[:], rc[:, 0:1], None, op0=OP.mult)
            scores_w[kind] = w_
            if dbg and kind == "ds":
                nc.sync.dma_start(dbg["d_wds"].ap(), w_[:].bitcast(f32))

        # broadcast weights down partitions, build ctx in place of topicrepT
        with tc.tile_pool(name="p6pb", bufs=1, space="PSUM") as p6pb:
            wbc = {}
            for kind in ("ds", "ts"):
                ps2 = [p6pb.tile([128, 512], f32, tag=f"wb{kind}{nh}", name=f"wb{kind}{nh}")
                       for nh in range(2)]
                for nh in range(2):
                    nc.tensor.matmul(ps2[nh][:], onesr_r[:, 0:128],
                                     scores_w[kind][:, nh * 512:(nh + 1) * 512],
                                     start=True, stop=True)
                wsb = p6w.tile([128, S * BD], f32r, tag=f"wsb{kind}", name=f"wsb{kind}")
                for nh in range(2):
                    nc.scalar.activation(wsb[:, nh * 512:(nh + 1) * 512], ps2[nh][:],
                                         AF.Identity)
                wbc[kind] = wsb
            for c in range(4):
                a = p6w.tile([128, S * BD], f32r, tag="ctxa", name="ctxa")
                for nh in range(2):
                    nc.vector.tensor_mul(a[:, nh * 512:(nh + 1) * 512],
                                         docrepT[c][:, nh * 512:(nh + 1) * 512],
                                         wbc["ds"][:, nh * 512:(nh + 1) * 512])
                nc.vector.tensor_mul(topicrepT[c][:], topicrepT[c][:], wbc["ts"][:])
                nc.vector.tensor_add(topicrepT[c][:], a[:], topicrepT[c][:])

        with tc.tile_pool(name="p6pd", bufs=2, space="PSUM") as p6pd, \
             tc.tile_pool(name="p6pl", bufs=1, space="PSUM") as p6pl:
            hdna = []
            for m2 in range(2):
                pm = [p6pd.tile([128, 512], f32, tag=f"dnap{nh}", name=f"dnap{nh}")
                      for nh in range(2)]
                for k in range(8):
                    rhs = srepT[k] if k < 4 else topicrepT[k - 4]
                    for nh in range(2):
                        nc.tensor.matmul(pm[nh][:],
                                         wdna[k][:, m2 * 128:(m2 + 1) * 128],
                                         rhs[:, nh * 512:(nh + 1) * 512],
                                         start=(k == 0), stop=(k == 7))
                hd = p6one.tile([128, H4], f32r, tag=f"hdna{m2}", name=f"hdna{m2}")
                for nh in range(2):
                    nc.scalar.activation(hd[:, nh * 512:(nh + 1) * 512], pm[nh][:],
                                         AF.Relu, bias=bdna[:, m2:m2 + 1])
                hdna.append(hd)

            lg_ps = [p6pl.tile([1, 512], f32, tag=f"lgp{nh}", name=f"lgp{nh}")
                     for nh in range(2)]
            for k2 in range(2):
                for nh in range(2):
                    nc.tensor.matmul(lg_ps[nh][:], wout[:, k2:k2 + 1],
                                     hdna[k2][:, nh * 512:(nh + 1) * 512],
                                     start=(k2 == 0), stop=(k2 == 1))
            lg = p6one.tile([1, S * BD], f32, tag="lg", name="lg")
            for nh in range(2):
                nc.scalar.activation(lg[:, nh * 512:(nh + 1) * 512], lg_ps[nh][:],
                                     AF.Identity, bias=bout[:, 0:1])
            nc.sync.dma_start(logits.ap(), lg[:])

    late_state["cm"].__exit__(None, None, None)
    ctx.close()


def _build():
    nc = bacc.Bacc("TRN2", target_bir_lowering=False, debug=False, num_devices=NC)
    ein = {}

    def inp(name, shape, dt=f32):
        ein[name] = nc.dram_tensor(name, shape, dt, kind="ExternalInput")

    inp("wid", [128, NW], i32)
    inp("emb", [V, E])
    inp("whhT_f", [H, H3]); inp("whhT_b", [H, H3])
    inp("wihT_f", [E + 1, H3]); inp("wihT_b", [E + 1, H3])
    inp("bhh_f", [H3]); inp("bhh_b", [H3])
    inp("w_att", [H4, H4]); inp("v_att", [H4, 1])
    inp("w_dna", [H4, D]); inp("b_dna", [D])
    inp("w_out", [D, 1]); inp("b_out", [1])
    inp("dvrows", [8, 1], i32)
    inp("tof", [4, 128], i32)
    inp("tenf", [128, 1]); inp("tepf", [128, 1])
    logits = nc.dram_tensor("logits", [1, S * BD], f32, kind="ExternalOutput")

    dbg = {}
    if int(os.environ.get("KDBG", "0")):
        for nm, shape in [("d_gif", [128, S * 24]), ("d_obf", [128, 8 * S]),
                          ("d_obb", [128, 8 * S]), ("d_srep", [BD * SP, H2]),
                          ("d_trep0", [128, S * BD]), ("d_wds", [1, S * BD]),
                          ("d_rz0", [128, 32]), ("d_nt0", [128, 16]),
                          ("d_t20", [128, 16]), ("d_om0", [128, 16])]:
            dbg[nm] = nc.dram_tensor(nm, shape, f32, kind="ExternalOutput")
    with tile.TileContext(nc) as tc:
        _emit(tc, nc, ein, logits, dbg)
    nc.compile()
    return nc


def _round_f32r(x):
    """Round mantissa so fp32r consumption is exact (keep 10 explicit bits)."""
    v = np.ascontiguousarray(x, dtype=np.float32).view(np.uint32).astype(np.uint64)
    v = (v + (1 << 12)) & np.uint64(0xFFFFE000)
    return v.astype(np.uint32).view(np.float32)


def _pack_core(c, word_ids, topic_start_ends, emb, Wih_f, Whh_f, bih_f, bhh_f,
               Wih_b, Whh_b, bih_b, bhh_b, W_att, v_att, W_dna, b_dna, W_out, b_out):
    f32c = lambda x: np.ascontiguousarray(x, dtype=np.float32)
    # wid: [128, ord*24+w]; partition p = s_loc*4 + d_loc; ord -> block map
    w = np.asarray(word_ids[c * BD:(c + 1) * BD])                  # [4, 256, 24]
    w = w.reshape(BD, NBLK, 32, L).transpose(1, 2, 0, 3)           # [blk, s_loc, d, w]
    wid = np.zeros((128, NW), np.int32)
    for o, b in enumerate(BLK_ORDER):
        wid[:, o * L:(o + 1) * L] = w[b].reshape(128, L)

    # wihT with 1/24 fold + bias row (bih + bhh for r,z; bih only for n)
    def wih_pack(Wih, bih, bhh):
        m = np.zeros((E + 1, H3), np.float32)
        m[0:E, :] = Wih.T / float(L)
        m[E, :] = bih
        m[E, 0:2 * H] += bhh[0:2 * H]
        return _round_f32r(m)

    dvrows = np.zeros((8, 1), np.int32)
    for d in range(BD):
        b = c * BD + d
        if b < 16:
            g0, g1 = 2 * b, 2 * b + 1
            rows = ((g0 // BD) * 8 + g0 % BD, (g1 // BD) * 8 + g1 % BD)
        else:
            g0, g1 = 2 * b - 32, 2 * b + 1 - 32
            rows = ((g0 // BD) * 8 + BD + g0 % BD, (g1 // BD) * 8 + BD + g1 % BD)
        dvrows[2 * d, 0], dvrows[2 * d + 1, 0] = rows

    tse = np.asarray(topic_start_ends[c * BD:(c + 1) * BD]).astype(np.int64)  # [4, T, 2]
    starts, ends = tse[..., 0], tse[..., 1]
    tof = np.zeros((4, 128), np.int32)
    tenf = np.zeros((128, 1), np.float32)
    tepf = np.zeros((128, 1), np.float32)
    for d in range(BD):
        b0 = d * 32
        tof[0, b0:b0 + T] = ends[d] + d * SP
        tof[1, b0:b0 + T] = np.maximum(starts[d] - 1, 0) + d * SP
        tof[2, b0:b0 + T] = starts[d] + d * SP
        tof[3, b0:b0 + T] = ends[d] + 1 + d * SP
        e = ends[d].astype(np.float64).copy()
        e[T - 1] += 1.0e9
        tenf[b0:b0 + T, 0] = e
        tepf[b0 + 1:b0 + T, 0] = e[0:T - 1]

    return {
        "wid": wid,
        "emb": f32c(emb),
        "whhT_f": f32c(Whh_f.T), "whhT_b": f32c(Whh_b.T),
        "wihT_f": wih_pack(Wih_f, bih_f, bhh_f),
        "wihT_b": wih_pack(Wih_b, bih_b, bhh_b),
        "bhh_f": f32c(bhh_f), "bhh_b": f32c(bhh_b),
        "w_att": _round_f32r(W_att), "v_att": _round_f32r(v_att),
        "w_dna": _round_f32r(W_dna), "b_dna": f32c(b_dna),
        "w_out": _round_f32r(W_out), "b_out": f32c(b_out),
        "dvrows": dvrows, "tof": tof, "tenf": tenf, "tepf": tepf,
    }


def kernel(**inputs):
    global _BUILT
    inputs = {k: np.asarray(v) for k, v in inputs.items()}
    if _BUILT is None:
        _BUILT = _build()
    nc = _BUILT
    in_maps = [_pack_core(c, **inputs) for c in range(NC)]
    res = run_bass_kernel_spmd(nc, in_maps, core_ids=list(range(NC)))
    out = np.zeros((B, S), np.float32)
    for c in range(NC):
        out[c * BD:(c + 1) * BD] = res.results[c]["logits"].reshape(S, BD).T
    return out

